# revision 37
# baseline (speedup 1.0000x reference)
"""CopyGenerator kernel for 8 trn2 NeuronCores.

Strategy (vocab tensor-parallel, fp8 DoubleRow, collective-free):
  - W's vocab dim (50000) is sharded 6250 cols/core, padded to 6272, scaled
    by 32 on the host and kept fp8e4 FULLY RESIDENT in SBUF (49KB/partition,
    loaded once at kernel start over the sync/scalar/gpsimd DMA queues as
    contiguous k-pair slabs in a partition-major host layout, the bytes
    row-tile 0 needs first).
  - Per core: logits = hidden @ (32*W_shard) via PE in fp8e4 DoubleRow
    (two 128-deep k-slices per matmul), fp32 PSUM accumulate over 4 k-pairs.
    Vocab cols are processed per 128-row tile in 4x 1536-wide groups that
    ping-pong two 3-bank PSUM slots plus a 128-wide tail group with its own
    1-bank double-buffered slot (2*3 + 2*1 = 8 banks). Within a group the
    stationary hidden block is loaded once per k-pair; a post-pass elides
    the redundant LDWEIGHTS so the PE streams back-to-back matmuls.
  - exp via ONE wide ACT activation per group (1536 cols spanning 3 PSUM
    banks) with scale=1/32 and fused row-sum accumulation (5 activations +
    5 accumulator reads per row tile). exp is written fp16 to SBUF and
    stored UNSCALED in four fp16 DMAs per row tile as the groups finish;
    per-core partial row-sums Z go out as a tiny second output.
  - The softmax denominator needs a cross-core sum; collectives on this
    platform cost 20-60us each with high jitter, so the kernel runs NO
    collectives at all: the host sums the 8 partial-Z tensors and folds
    (1-copy)/Z into the unsharded output in fp32 (one elementwise multiply
    over the gathered [N, V] result, alongside the host-computed copy gate
    sigmoid and the tiny attn x src_map einsum). Device critical path is
    pure load -> matmul -> exp -> store with no cross-core dependency.
PAD col handling: host zeroes W[:,1] on core 0, kernel masks the exp column
(cmask) and the host subtracts the leftover exp(0)=1 from that core's Z.
Output is fp16 on device; host upcasts to fp32.
"""

import numpy as np

N, D, V = 2048, 1024, 50000
S, B, CV = 100, 32, 120
NCORES = 8
VREAL = 6250          # real vocab cols per core
VPAD = 6272           # padded (12*512 + 128)
KT = 8                # k-tiles of 128 over D
JP = KT // 2          # DoubleRow k-pairs
NT = N // 128         # 16 row tiles
# col groups: (col offset, matmul width, exp/accum width). The 1536-wide
# groups ping-pong two 3-bank PSUM slots; the 128-wide tail group has its
# own 1-bank double-buffered slot. 2*3 + 2*1 = 8 PSUM banks.
GROUPS = [(0, 1536, 1536), (1536, 1536, 1536), (3072, 1536, 1536),
          (4608, 1536, 1536), (6144, 128, 106)]
NG = len(GROUPS)
PAD_IDX = 1
WSCALE = 32.0

_CACHE = {}
TRACE = False


def _install_walrus_compat():
    """This container's walrus build rejects >1 sync-wait per instruction.
    Patch the Tile drain to chain single-wait drains, and provide a module
    post-pass hoisting extra waits onto same-engine NoOps."""
    import concourse.tile as tile_mod
    import concourse.mybir as mybir
    from concourse.vector_clock import ScopedClock

    if getattr(tile_mod.TileContext._drain_and_barrier, "_waitsplit", False):
        return

    def _patched_drain_and_barrier(self, tick_clock, wait_clock):
        nc = self.nc
        drain_inst = nc.sync.drain()
        wait_clock.add_sem_waits(
            drain_inst.ins, ScopedClock({None: tick_clock.global_clock})
        )
        si = drain_inst.ins.sync_info
        waits = list(si.on_wait) if si and si.on_wait else []
        if len(waits) > 1:
            si.on_wait = waits[:1]
            rest = waits[1:]
            while rest:
                chunk, rest = rest[:1], rest[1:]
                d2 = nc.sync.drain()
                if d2.ins.sync_info is None:
                    d2.ins.sync_info = mybir.SyncInfo(on_wait=chunk, on_update=[])
                else:
                    d2.ins.sync_info.on_wait = chunk
        nc.all_engine_barrier()
        assert self.sems is not None
        popped = nc._tile_sem_poison_stack.pop()
        assert popped is self._sem_poison
        nc.clear_and_free_semaphores(list(self.sems.allocated().values()))
        nc.all_engine_barrier()

    _patched_drain_and_barrier._waitsplit = True
    tile_mod.TileContext._drain_and_barrier = _patched_drain_and_barrier


def _elide_redundant_ldweights(nc):
    """Delete InstLdweights whose weights AP is identical to the previous
    PE weight load (only InstMatmult/InstNoOp between): the PE array keeps
    the stationary operand across matmuls. A redundant load carrying sync
    info is turned into a PE NoOp instead of being deleted."""
    import concourse.mybir as mybir

    n_elided = 0
    for fn in nc.m.functions:
        for bb in fn.blocks:
            last_sig = None
            new = []
            changed = False
            for ins in bb.instructions:
                tn = type(ins).__name__
                if getattr(ins, "engine", None) == mybir.EngineType.PE:
                    if tn == "InstLdweights":
                        sig = str(ins.ins[0])
                        if sig == last_sig:
                            changed = True
                            n_elided += 1
                            si = ins.sync_info
                            has_sync = si and (si.on_wait or si.on_update)
                            if has_sync:
                                new.append(
                                    mybir.InstNoOp(
                                        name=f"I-ldwelide-{ins.name}",
                                        sync_info=si,
                                        bass_nofuse=True,
                                        engine=ins.engine,
                                    )
                                )
                            continue
                        last_sig = sig
                    elif tn not in ("InstMatmult", "InstNoOp"):
                        last_sig = None
                new.append(ins)
            if changed:
                bb.instructions[:] = new
    return n_elided


def _split_multi_waits(nc):
    import concourse.mybir as mybir

    uid = 0
    n_split = 0
    for fn in nc.m.functions:
        for bb in fn.blocks:
            old = list(bb.instructions)
            new = []
            changed = False
            for ins in old:
                si = ins.sync_info
                waits = list(si.on_wait) if si and si.on_wait else []
                if len(waits) > 1:
                    changed = True
                    n_split += 1
                    for w in waits[:-1]:
                        uid += 1
                        new.append(
                            mybir.InstNoOp(
                                name=f"I-waitsplit-{uid}-{ins.name}",
                                sync_info=mybir.SyncInfo(on_wait=[w], on_update=[]),
                                bass_nofuse=True,
                                engine=ins.engine,
                            )
                        )
                    si.on_wait = [waits[-1]]
                new.append(ins)
            if changed:
                bb.instructions[:] = new
    return n_split


def _build_nc():
    import concourse.bass as bass
    import concourse.mybir as mybir
    import concourse.tile as tile

    _install_walrus_compat()

    f32 = mybir.dt.float32
    f16 = mybir.dt.float16
    f8 = mybir.dt.float8e4
    AF = mybir.ActivationFunctionType
    OP = mybir.AluOpType
    AX = mybir.AxisListType
    DR = mybir.MatmulPerfMode.DoubleRow

    nc = bass.Bass()
    # W8/hT8 arrive pre-permuted to partition-major [128, KT, cols] so each
    # column-group load is wide contiguous bursts instead of 512B lines
    hT8 = nc.dram_tensor("hT8", [128, KT, N], f8, kind="ExternalInput")
    W8 = nc.dram_tensor("W8", [128, KT, VPAD], f8, kind="ExternalInput")
    cmask = nc.dram_tensor("cmask", [128, 2], f16, kind="ExternalInput")
    out = nc.dram_tensor("out", [N, VREAL], f16, kind="ExternalOutput")
    zpart = nc.dram_tensor("zpart", [128, NT], f32, kind="ExternalOutput")

    with tile.TileContext(nc) as tc:
        with (
            tc.tile_pool(name="expp", bufs=6) as expp,
            tc.tile_pool(name="zpp", bufs=4) as zpp,
            tc.tile_pool(name="smallp", bufs=1) as smallp,
            tc.tile_pool(name="psmain", bufs=2, space="PSUM") as psmain,
        ):
            # Load order tuned for fast start: W loads are k-pair slabs
            # ([:, 2j:2j+2, :] = one contiguous 12.5KB burst per partition
            # in the partition-major dram layout), spread over the three
            # DMA-capable queues in the order row-tile 0's j-loop consumes
            # them. hT's first row-tile cols land first on gpsimd.
            wres = smallp.tile([128, KT, VPAD], f8)
            htres = smallp.tile([128, KT, N], f8)
            cmask_sb = smallp.tile([128, 2], f16)
            c0cols = 512

            def wload(q, j, a, b):
                q.dma_start(wres[:, 2 * j : 2 * j + 2, a:b],
                            W8[0:128, 2 * j : 2 * j + 2, a:b])

            # priority bytes first: row-tile 0's group A consumes cols
            # 0:1536 of every k-slab within its first ~3us of matmuls
            nc.gpsimd.dma_start(htres[:, :, 0:c0cols], hT8[0:128, 0:KT, 0:c0cols])
            nc.scalar.dma_start(cmask_sb[:], cmask[:])
            wload(nc.sync, 0, 0, 1536)
            wload(nc.scalar, 1, 0, 1536)
            wload(nc.gpsimd, 2, 0, 1536)
            wload(nc.sync, 3, 0, 1536)
            wload(nc.scalar, 0, 1536, VPAD)
            wload(nc.gpsimd, 1, 1536, VPAD)
            wload(nc.sync, 2, 1536, VPAD)
            wload(nc.scalar, 3, 1536, VPAD)
            nc.gpsimd.dma_start(htres[:, :, c0cols:N], hT8[0:128, 0:KT, c0cols:N])

            zsum = smallp.tile([128, NT], f32)

            for t in range(NT):
                rs = t * 128
                ex = expp.tile([128, VREAL], f16, tag="exp", name=f"exp{t}")
                zp = zpp.tile([128, NG], f32, tag="zpart", name=f"zp{t}")
                for gi, (goff, mw, ew) in enumerate(GROUPS):
                    if mw >= 512:
                        pm = psmain.tile(
                            [128, 1536], f32, tag="pg", name=f"pm{t}_{gi}"
                        )
                    else:
                        pm = psmain.tile([128, 128], f32, tag="pd", name=f"pmd{t}")
                    nsub = (mw + 511) // 512
                    for j in range(JP):
                        for si in range(nsub):
                            sw = min(512, mw - si * 512)
                            nc.tensor.matmul(
                                pm[:, si * 512 : si * 512 + sw],
                                htres[:, 2 * j : 2 * j + 2, rs : rs + 128],
                                wres[:, 2 * j : 2 * j + 2,
                                     goff + si * 512 : goff + si * 512 + sw],
                                start=(j == 0),
                                stop=(j == JP - 1),
                                perf_mode=DR,
                            )
                    nc.scalar.activation(
                        ex[:, goff : goff + ew], pm[:, 0:ew], AF.Exp,
                        scale=1.0 / WSCALE,
                        accum_out=zp[:, gi : gi + 1],
                    )
                    if gi == 0:
                        # zero masked cols (PAD on core 0; all-ones elsewhere)
                        nc.vector.tensor_tensor(
                            ex[:, 0:2], ex[:, 0:2], cmask_sb[:], OP.mult
                        )
                    if gi == 1:
                        # store columns as soon as their groups land; host
                        # normalizes (unscaled exp out)
                        nc.sync.dma_start(
                            out[rs : rs + 128, 0:3072], ex[:, 0:3072]
                        )
                    elif gi == 2:
                        nc.sync.dma_start(
                            out[rs : rs + 128, 3072:4608], ex[:, 3072:4608]
                        )
                    elif gi == 3:
                        nc.sync.dma_start(
                            out[rs : rs + 128, 4608:6144], ex[:, 4608:6144]
                        )
                nc.vector.tensor_reduce(
                    zsum[:, t : t + 1], zp[:, :], axis=AX.X, op=OP.add
                )
                nc.sync.dma_start(out[rs : rs + 128, 6144:VREAL], ex[:, 6144:VREAL])
            nc.gpsimd.dma_start(zpart[:, :], zsum[:, :])

    _elide_redundant_ldweights(nc)
    _split_multi_waits(nc)
    return nc


def _get_nc():
    if "nc" not in _CACHE:
        _CACHE["nc"] = _build_nc()
    return _CACHE["nc"]


def kernel(**inputs):
    import ml_dtypes
    from concourse.bass_utils import run_bass_kernel_spmd

    f8np = ml_dtypes.float8_e4m3

    hidden = np.asarray(inputs["hidden"], np.float32)
    attn = np.asarray(inputs["attn"], np.float32)
    src_map = np.asarray(inputs["src_map"], np.float32)
    W = np.asarray(inputs["W"], np.float32)
    w_copy = np.asarray(inputs["w_copy"], np.float32)
    b_copy = np.asarray(inputs["b_copy"], np.float32)

    nc = _get_nc()

    # partition-major [128, KT, cols] device layouts (see _build_nc)
    hT8_h = np.ascontiguousarray(
        hidden.T.astype(f8np).reshape(KT, 128, N).transpose(1, 0, 2)
    )

    in_maps = []
    for cc in range(NCORES):
        Wc = np.zeros((D, VPAD), f8np)
        Wcf = W[:, cc * VREAL : (cc + 1) * VREAL] * WSCALE
        if cc == 0:
            Wcf = Wcf.copy()
            Wcf[:, PAD_IDX] = 0.0
        Wc[:, :VREAL] = Wcf.astype(f8np)
        Wc = np.ascontiguousarray(Wc.reshape(KT, 128, VPAD).transpose(1, 0, 2))
        cm = np.ones((128, 2), np.float16)
        if cc == 0:
            cm[:, PAD_IDX] = 0.0
        in_maps.append({"hT8": hT8_h, "W8": Wc, "cmask": cm})

    res = run_bass_kernel_spmd(nc, in_maps, list(range(NCORES)), trace=TRACE)
    _CACHE["last_result"] = res

    # ---- host epilogue: copy gate, softmax normalization, copy path ----
    c = 1.0 / (1.0 + np.exp(-(hidden @ w_copy + b_copy)))      # [N, 1] f32
    omc = (1.0 - c[:, 0]).astype(np.float32)                   # [N]

    # global Z: sum the 8 per-core partial row-sums; drop core 0's masked
    # PAD column contribution exp(0)=1 (accum runs before the cmask zeroing)
    zs = np.zeros(N, np.float32)
    for r in res.results:
        zs += np.asarray(r["zpart"]).T.reshape(N)              # [NT,128]->[N]
    zs -= 1.0
    scale = (omc / zs).astype(np.float32)                      # [N]

    full = np.empty((N, V + CV), np.float32)
    for cc, r in enumerate(res.results):
        full[:, cc * VREAL : (cc + 1) * VREAL] = r["out"]
    full[:, :V] *= scale[:, None]

    # copy path: copy_prob = einsum(attn*c, src_map)  [N, CV]
    mul_attn = (attn * c).reshape(-1, B, S).transpose(1, 0, 2)   # [B, T, S]
    smap_b = src_map.transpose(1, 0, 2)                          # [B, S, CV]
    copy_prob = np.matmul(mul_attn, smap_b)                      # [B, T, CV]
    full[:, V:] = copy_prob.transpose(1, 0, 2).reshape(-1, CV)
    return full
